# revision 1
# baseline (speedup 1.0000x reference)
"""Trainium2 Bass kernel for nn_CM_sampler (retrieval_knn).

Computes, for each of 10000 class-blocked representatives (10 classes x 1000),
the number of other-class representatives within euclidean distance 0.5
(via the gram trick: d2 = sq_i + sq_j - 2*X@X.T, count d2 < 0.25), then
selects per class the budget//C lowest-count rows (host-side argsort).

Sharding: the 10000 query rows are split across 8 cores; each core gets one
125-row query tile from EACH of the 10 classes (queries
[cls*1000 + core*125, cls*1000 + (core+1)*125) for cls in 0..9), so the
same-class-skip pattern is identical on every core and one SPMD program
serves all 8. Keys (all 10000) are streamed in 20 chunks of 500; the two
chunks belonging to a query tile's class are skipped (the reference only
counts other-class neighbors).

Device math per (query tile q of 125, key chunk of 500):
  PSUM  = 2*Q @ K^T - sq_k[None, :]      (8 K=128 matmuls + 1 augmented K=1)
  count += sum_j [ PSUM > (sq_q - 0.25) ]  (one fused DVE tensor_scalar
                                            is_gt + row-accumulate)
which is exactly d2 < 0.25 up to fp rounding-order.
"""

import numpy as np

C, P, D = 10, 1000, 1024
N = C * P  # 10000
N_CORES = 8
QT = 125  # query tile rows (divides 1000; 8 tiles per class -> 1 per core)
NQT = C  # query tiles per core (one per class)
QPC = QT * NQT  # 1250 queries per core
KC = 500  # key chunk (matmul free dim)
NKC = N // KC  # 20 chunks; chunk c belongs to class c//2
KS = D // 128  # 8 contraction slices

_PROG_CACHE = {}


def _build_program(mm_mode):
    """Build + compile the SPMD Bass program. mm_mode: 'f32' | 'f32r' | 'f32r2'."""
    import concourse.mybir as mybir
    from concourse import bacc
    from concourse.tile import TileContext

    f32 = mybir.dt.float32
    if mm_mode == "f32":
        mm_dt = f32
        n_pass = 1
    elif mm_mode == "f32r":
        mm_dt = mybir.dt.float32r
        n_pass = 1
    elif mm_mode == "f32r2":
        mm_dt = mybir.dt.float32r
        n_pass = 2
    else:
        raise ValueError(mm_mode)

    nc = bacc.Bacc("TRN2", target_bir_lowering=False, debug=False, num_devices=N_CORES)

    # Per-core inputs. qT holds 2*Q^T (the 2x is folded into the stationary
    # operand so PSUM accumulates 2G directly); for 2-pass f32r, qT2 holds the
    # residual of qT after the hardware's high-half truncation.
    qT = nc.dram_tensor("qT", [KS, 128, QPC], f32, kind="ExternalInput").ap()
    qT2 = None
    if n_pass == 2:
        qT2 = nc.dram_tensor("qT2", [KS, 128, QPC], f32, kind="ExternalInput").ap()
    kT = nc.dram_tensor("kT", [KS, 128, N], f32, kind="ExternalInput").ap()
    kaug = nc.dram_tensor("kaug", [1, N], f32, kind="ExternalInput").ap()
    qthr = nc.dram_tensor("qthr", [QT, NQT], f32, kind="ExternalInput").ap()
    cnt_out = nc.dram_tensor("cnt", [QT, NQT], f32, kind="ExternalOutput").ap()

    with TileContext(nc) as tc:
        with (
            tc.tile_pool(name="qpool", bufs=1) as qpool,
            tc.tile_pool(name="kpool", bufs=3) as kpool,
            tc.tile_pool(name="small", bufs=1) as spool,
            tc.tile_pool(name="psum", bufs=4, space="PSUM") as pspool,
        ):
            # Resident stationary operands: 2*Q^T k-slices (+ residuals).
            qt_tiles = []
            for ks in range(KS):
                t = qpool.tile([128, QPC], mm_dt, tag=f"q{ks}")
                nc.sync.dma_start(out=t[:], in_=qT[ks])
                qt_tiles.append(t)
            qt2_tiles = []
            if n_pass == 2:
                for ks in range(KS):
                    t = qpool.tile([128, QPC], mm_dt, tag=f"q2_{ks}")
                    nc.sync.dma_start(out=t[:], in_=qT2[ks])
                    qt2_tiles.append(t)

            qthr_t = spool.tile([QT, NQT], f32, tag="qthr")
            nc.sync.dma_start(out=qthr_t[:], in_=qthr[:])
            ones_t = spool.tile([1, QT], mm_dt, tag="ones")
            nc.vector.memset(ones_t[:], 1.0)

            acc = spool.tile([QT, NQT * 18], f32, tag="acc")
            scratch = spool.tile([QT, KC], f32, tag="scratch")
            cnt_t = spool.tile([QT, NQT], f32, tag="cnt")

            col = [0] * NQT  # next acc column per query tile
            for c in range(NKC):
                kt = kpool.tile([128, KS, KC], mm_dt, tag="kt")
                for ks in range(KS):
                    nc.sync.dma_start(
                        out=kt[:, ks, :], in_=kT[ks, :, c * KC : (c + 1) * KC]
                    )
                ka = kpool.tile([1, KC], mm_dt, tag="ka")
                nc.sync.dma_start(out=ka[:], in_=kaug[:, c * KC : (c + 1) * KC])

                for t in range(NQT):
                    if t == c // 2:
                        continue  # same-class chunk: reference skips class t
                    ps = pspool.tile([QT, KC], f32)
                    for ks in range(KS):
                        nc.tensor.matmul(
                            ps[:],
                            qt_tiles[ks][:, t * QT : (t + 1) * QT],
                            kt[:, ks, :],
                            start=(ks == 0),
                            stop=False,
                        )
                    if n_pass == 2:
                        for ks in range(KS):
                            nc.tensor.matmul(
                                ps[:],
                                qt2_tiles[ks][:, t * QT : (t + 1) * QT],
                                kt[:, ks, :],
                                start=False,
                                stop=False,
                            )
                    # augmented row: ones^T @ (-sq_k) accumulates -sq_j
                    nc.tensor.matmul(ps[:], ones_t[:], ka[:], start=False, stop=True)
                    # count_j [ps > sq_q - 0.25], accumulated along the free axis
                    import concourse.mybir as mb

                    nc.vector.tensor_scalar(
                        out=scratch[:],
                        in0=ps[:],
                        scalar1=qthr_t[:, t : t + 1],
                        scalar2=None,
                        op0=mb.AluOpType.is_gt,
                        op1=mb.AluOpType.add,
                        accum_out=acc[:, t * 18 + col[t] : t * 18 + col[t] + 1],
                    )
                    col[t] += 1

            for t in range(NQT):
                nc.vector.tensor_reduce(
                    cnt_t[:, t : t + 1],
                    acc[:, t * 18 : (t + 1) * 18],
                    axis=mybir.AxisListType.X,
                    op=mybir.AluOpType.add,
                )
            nc.sync.dma_start(out=cnt_out[:], in_=cnt_t[:])

    nc.compile()
    return nc


def _get_program(mm_mode):
    if mm_mode not in _PROG_CACHE:
        _PROG_CACHE[mm_mode] = _build_program(mm_mode)
    return _PROG_CACHE[mm_mode]


def _f32r_high(w):
    """Model of the PE's fp32 'High' weight truncation: upper 16 bits."""
    u = w.view(np.uint32) & np.uint32(0xFFFF0000)
    return u.view(np.float32)


MM_MODE = "f32"


def _prepare_inputs(X, sq):
    """Build per-core in_maps from X [N, D] f32 and sq [N] f32."""
    kT_full = np.ascontiguousarray(X.T.reshape(KS, 128, N))
    kaug_full = np.ascontiguousarray((-sq).reshape(1, N))

    in_maps = []
    for core in range(N_CORES):
        rows = np.concatenate(
            [
                np.arange(cls * P + core * QT, cls * P + (core + 1) * QT)
                for cls in range(C)
            ]
        )
        Q2 = 2.0 * X[rows]  # [QPC, D], exact scaling
        qT_c = np.ascontiguousarray(Q2.T.reshape(KS, 128, QPC))
        qthr_c = np.ascontiguousarray(
            (sq[rows] - np.float32(0.25)).reshape(NQT, QT).T
        )
        m = {
            "qT": qT_c,
            "kT": kT_full,
            "kaug": kaug_full,
            "qthr": qthr_c,
        }
        if MM_MODE == "f32r2":
            m["qT"] = _f32r_high(qT_c)
            m["qT2"] = np.ascontiguousarray(qT_c - m["qT"])
        in_maps.append(m)
    return in_maps


def _counts_from_results(results):
    counts = np.zeros(N, dtype=np.int64)
    for core in range(N_CORES):
        out = results[core]["cnt"]  # [QT, NQT] f32
        for cls in range(C):
            counts[cls * P + core * QT : cls * P + (core + 1) * QT] = out[
                :, cls
            ].astype(np.int64)
    return counts


def kernel(feats, ids_per_cls, budget, _bench=None):
    from concourse.bass_utils import run_bass_kernel_spmd

    feats = np.asarray(feats, dtype=np.float32)
    ids_per_cls = np.asarray(ids_per_cls)
    budget_i = int(np.asarray(budget))

    ids_flat = ids_per_cls.reshape(-1).astype(np.int64)
    X = np.ascontiguousarray(feats[ids_flat])  # [N, D] class-blocked
    # fp32 row sum-of-squares (computed in f64, rounded once)
    sq = (X.astype(np.float64) ** 2).sum(axis=1).astype(np.float32)

    nc = _get_program(MM_MODE)
    in_maps = _prepare_inputs(X, sq)
    kw = dict(_bench) if _bench else {}
    res = run_bass_kernel_spmd(nc, in_maps, core_ids=list(range(N_CORES)), **kw)
    counts = _counts_from_results(res.results)

    counts = counts.reshape(C, P)
    per_cls_budget = budget_i // C
    order = np.argsort(counts, axis=-1, kind="stable")
    sel = order[:, :per_cls_budget]
    ids_selected = np.take_along_axis(
        ids_per_cls.reshape(C, P), sel, axis=1
    ).reshape(-1)

    counts_out = counts.astype(np.int32)
    if _bench is not None:
        return (ids_selected, counts_out), res
    return ids_selected, counts_out


# revision 3
# speedup vs baseline: 1.2853x; 1.2853x over previous
"""Trainium2 Bass kernel for nn_CM_sampler (retrieval_knn).

Computes, for each of 10000 class-blocked representatives (10 classes x 1000),
the number of other-class representatives within euclidean distance 0.5
(gram trick: d2 = sq_i + sq_j - 2*X@X.T, count d2 < 0.25), then selects per
class the budget//C lowest-count rows (host-side argsort, tiny).

Sharding: the 10000 query rows are split across 8 cores; each core gets one
125-row query tile from EACH of the 10 classes (queries
[cls*1000 + core*125, cls*1000 + (core+1)*125)), so the same-class-skip
pattern is identical on every core and one SPMD program serves all 8.
Keys (all 10000) are streamed in 20 chunks of 500; the two chunks of a query
tile's own class are skipped (reference counts other-class neighbors only).

Device math per (query tile q of 125 rows, key chunk of 500):
  PSUM  = 2*Q @ K^T - sq_k[None, :]     (matmuls + augmented rows)
  count += sum_j [ PSUM > (sq_q - 0.25) ]   (one fused DVE tensor_scalar
                                             is_gt + row-accumulate)
which is d2 < 0.25 up to fp rounding placement.

Matmul modes:
  f32   - plain fp32 matmuls (4 cycles/row on TensorE).
  f32r  - TF32-like single pass (1 cycle/row): HW rounds both operands to
          ~11-bit mantissas (RNE); d2 error ~2e-6.
  f32r3 - 3-pass split w_h@x_h + w_h@x_l + w_l@x_h with hi = RNE-at-13-bits
          (grid-aligned so the HW's ~12-bit rounding passes hi through
          exactly; 11x11-bit products are exact in fp32) -> fp32-grade
          precision at 3 cycles/row.
The -sq_k augmented rows are always fed as hi+residual pairs so they are
exact under f32r rounding.
"""

import numpy as np

C, P, D = 10, 1000, 1024
N = C * P  # 10000
N_CORES = 8
QT = 125  # query tile rows (divides 1000; 8 tiles per class -> 1 per core)
NQT = C  # query tiles per core (one per class)
QPC = QT * NQT  # 1250 queries per core
KC = 500  # key chunk (matmul free dim)
NKC = N // KC  # 20 chunks; chunk c belongs to class c//2
KS = D // 128  # 8 contraction slices

MM_MODE = "f32r3"

_PROG_CACHE = {}


def _rne(a, k):
    """Round fp32 array to (23-k) explicit mantissa bits, round-to-nearest-even."""
    u = a.view(np.uint32).astype(np.uint64)
    bias = ((u >> k) & 1) + np.uint64((1 << (k - 1)) - 1)
    r = (((u + bias) >> k) << k) & np.uint64(0xFFFFFFFF)
    return r.astype(np.uint32).view(np.float32)


def _build_program(mm_mode):
    import concourse.mybir as mybir
    from concourse import bacc
    from concourse.tile import TileContext

    f32 = mybir.dt.float32
    mm_dt = f32 if mm_mode == "f32" else mybir.dt.float32r
    split = mm_mode == "f32r3"
    n_aug = 1 if mm_mode == "f32" else 2

    nc = bacc.Bacc("TRN2", target_bir_lowering=False, debug=False, num_devices=N_CORES)

    # qT holds 2*Q^T k-slices (2x folded into the stationary operand so PSUM
    # accumulates 2G directly). In split mode qT/kT are the hi parts and
    # qTl/kTl the residuals.
    qT = nc.dram_tensor("qT", [KS, 128, QPC], mm_dt, kind="ExternalInput").ap()
    kT = nc.dram_tensor("kT", [KS, 128, N], mm_dt, kind="ExternalInput").ap()
    if split:
        qTl = nc.dram_tensor("qTl", [KS, 128, QPC], mm_dt, kind="ExternalInput").ap()
        kTl = nc.dram_tensor("kTl", [KS, 128, N], mm_dt, kind="ExternalInput").ap()
    kaug = nc.dram_tensor("kaug", [n_aug, N], mm_dt, kind="ExternalInput").ap()
    onesw = nc.dram_tensor("onesw", [n_aug, QT], mm_dt, kind="ExternalInput").ap()
    qthr = nc.dram_tensor("qthr", [QT, NQT], f32, kind="ExternalInput").ap()
    cnt_out = nc.dram_tensor("cnt", [QT, NQT], f32, kind="ExternalOutput").ap()

    with TileContext(nc) as tc:
        with (
            tc.tile_pool(name="qpool", bufs=1) as qpool,
            tc.tile_pool(name="kpool", bufs=2 if split else 3) as kpool,
            tc.tile_pool(name="small", bufs=1) as spool,
            tc.tile_pool(name="psum", bufs=4, space="PSUM") as pspool,
        ):
            qt_tiles = []
            qtl_tiles = []
            for ks in range(KS):
                t = qpool.tile([128, QPC], mm_dt, tag=f"q{ks}")
                nc.sync.dma_start(out=t[:], in_=qT[ks])
                qt_tiles.append(t)
                if split:
                    tl = qpool.tile([128, QPC], mm_dt, tag=f"ql{ks}")
                    nc.sync.dma_start(out=tl[:], in_=qTl[ks])
                    qtl_tiles.append(tl)

            qthr_t = spool.tile([QT, NQT], f32, tag="qthr")
            nc.sync.dma_start(out=qthr_t[:], in_=qthr[:])
            ones_t = spool.tile([n_aug, QT], mm_dt, tag="ones")
            nc.sync.dma_start(out=ones_t[:], in_=onesw[:])

            acc = spool.tile([QT, NQT * 18], f32, tag="acc")
            scratch = spool.tile([QT, KC], f32, tag="scratch")
            cnt_t = spool.tile([QT, NQT], f32, tag="cnt")

            col = [0] * NQT
            for c in range(NKC):
                kt = kpool.tile([128, KS, KC], mm_dt, tag="kt")
                for ks in range(KS):
                    nc.sync.dma_start(
                        out=kt[:, ks, :], in_=kT[ks, :, c * KC : (c + 1) * KC]
                    )
                if split:
                    ktl = kpool.tile([128, KS, KC], mm_dt, tag="ktl")
                    for ks in range(KS):
                        nc.sync.dma_start(
                            out=ktl[:, ks, :], in_=kTl[ks, :, c * KC : (c + 1) * KC]
                        )
                ka = kpool.tile([n_aug, KC], mm_dt, tag="ka")
                nc.sync.dma_start(out=ka[:], in_=kaug[:, c * KC : (c + 1) * KC])

                for t in range(NQT):
                    if t == c // 2:
                        continue  # same-class chunk: reference skips class t
                    ps = pspool.tile([QT, KC], f32)
                    for ks in range(KS):
                        nc.tensor.matmul(
                            ps[:],
                            qt_tiles[ks][:, t * QT : (t + 1) * QT],
                            kt[:, ks, :],
                            start=(ks == 0),
                            stop=False,
                        )
                    if split:
                        for ks in range(KS):
                            nc.tensor.matmul(
                                ps[:],
                                qt_tiles[ks][:, t * QT : (t + 1) * QT],
                                ktl[:, ks, :],
                                start=False,
                                stop=False,
                            )
                        for ks in range(KS):
                            nc.tensor.matmul(
                                ps[:],
                                qtl_tiles[ks][:, t * QT : (t + 1) * QT],
                                kt[:, ks, :],
                                start=False,
                                stop=False,
                            )
                    # augmented row(s): ones^T @ (-sq_k hi/lo) accumulate -sq_j
                    nc.tensor.matmul(ps[:], ones_t[:], ka[:], start=False, stop=True)
                    # count_j [ps > sq_q - 0.25] accumulated along the free axis
                    nc.vector.tensor_scalar(
                        out=scratch[:],
                        in0=ps[:],
                        scalar1=qthr_t[:, t : t + 1],
                        scalar2=None,
                        op0=mybir.AluOpType.is_gt,
                        op1=mybir.AluOpType.add,
                        accum_out=acc[:, t * 18 + col[t] : t * 18 + col[t] + 1],
                    )
                    col[t] += 1

            for t in range(NQT):
                nc.vector.tensor_reduce(
                    cnt_t[:, t : t + 1],
                    acc[:, t * 18 : (t + 1) * 18],
                    axis=mybir.AxisListType.X,
                    op=mybir.AluOpType.add,
                )
            nc.sync.dma_start(out=cnt_out[:], in_=cnt_t[:])

    nc.compile()
    return nc


def _get_program(mm_mode):
    if mm_mode not in _PROG_CACHE:
        _PROG_CACHE[mm_mode] = _build_program(mm_mode)
    return _PROG_CACHE[mm_mode]


def _prepare_inputs(X, sq, mm_mode):
    """Build per-core in_maps from X [N, D] f32 and sq [N] f32."""
    split = mm_mode == "f32r3"

    kT_full = np.ascontiguousarray(X.T.reshape(KS, 128, N))
    if split:
        kT_hi = _rne(kT_full, 13)
        kT_lo = np.ascontiguousarray(kT_full - kT_hi)
        kT_hi = np.ascontiguousarray(kT_hi)

    msq = (-sq).astype(np.float32)
    if mm_mode == "f32":
        kaug_full = msq.reshape(1, N).copy()
    else:
        hi = _rne(msq, 13)
        kaug_full = np.ascontiguousarray(np.stack([hi, msq - hi], axis=0))

    in_maps = []
    for core in range(N_CORES):
        rows = np.concatenate(
            [
                np.arange(cls * P + core * QT, cls * P + (core + 1) * QT)
                for cls in range(C)
            ]
        )
        Q2 = 2.0 * X[rows]  # exact scaling
        qT_c = np.ascontiguousarray(Q2.T.reshape(KS, 128, QPC))
        qthr_c = np.ascontiguousarray(
            (sq[rows] - np.float32(0.25)).reshape(NQT, QT).T
        )
        m = {
            "kT": kT_full,
            "kaug": kaug_full,
            "qthr": qthr_c,
            "onesw": np.ones((kaug_full.shape[0], QT), dtype=np.float32),
        }
        if split:
            qhi = _rne(qT_c, 13)
            m["qT"] = np.ascontiguousarray(qhi)
            m["qTl"] = np.ascontiguousarray(qT_c - qhi)
            m["kT"] = kT_hi
            m["kTl"] = kT_lo
        else:
            m["qT"] = qT_c
        in_maps.append(m)
    return in_maps


def _counts_from_results(results):
    counts = np.zeros(N, dtype=np.int64)
    for core in range(N_CORES):
        out = results[core]["cnt"]  # [QT, NQT] f32
        for cls in range(C):
            counts[cls * P + core * QT : cls * P + (core + 1) * QT] = out[
                :, cls
            ].astype(np.int64)
    return counts


def kernel(feats, ids_per_cls, budget, _bench=None):
    from concourse.bass_utils import run_bass_kernel_spmd

    feats = np.asarray(feats, dtype=np.float32)
    ids_per_cls = np.asarray(ids_per_cls)
    budget_i = int(np.asarray(budget))

    ids_flat = ids_per_cls.reshape(-1).astype(np.int64)
    X = np.ascontiguousarray(feats[ids_flat])  # [N, D] class-blocked
    sq = (X.astype(np.float64) ** 2).sum(axis=1).astype(np.float32)

    nc = _get_program(MM_MODE)
    in_maps = _prepare_inputs(X, sq, MM_MODE)
    kw = dict(_bench) if _bench else {}
    res = run_bass_kernel_spmd(nc, in_maps, core_ids=list(range(N_CORES)), **kw)
    counts = _counts_from_results(res.results)

    counts = counts.reshape(C, P)
    per_cls_budget = budget_i // C
    order = np.argsort(counts, axis=-1, kind="stable")
    sel = order[:, :per_cls_budget]
    ids_selected = np.take_along_axis(
        ids_per_cls.reshape(C, P), sel, axis=1
    ).reshape(-1)

    counts_out = counts.astype(np.int32)
    if _bench is not None:
        return (ids_selected, counts_out), res
    return ids_selected, counts_out


# revision 4
# speedup vs baseline: 3.1263x; 2.4324x over previous
"""Trainium2 Bass kernel for nn_CM_sampler (retrieval_knn).

Computes, for each of 10000 class-blocked representatives (10 classes x 1000),
the number of other-class representatives within euclidean distance 0.5
(gram trick: d2 = sq_i + sq_j - 2*X@X.T, count d2 < 0.25), then selects per
class the budget//C lowest-count rows (host-side argsort, tiny).

Sharding: the 10000 query rows are split across 8 cores; each core gets one
125-row query tile from EACH of the 10 classes (queries
[cls*1000 + core*125, cls*1000 + (core+1)*125)), so the same-class-skip
pattern is identical on every core and one SPMD program serves all 8.
Keys (all 10000) are streamed in 20 chunks of 500; the two chunks of a query
tile's own class are skipped (reference counts other-class neighbors only).

Device math per (query tile q of 125 rows, key chunk of 500):
  PSUM  = 2*Q @ K^T - sq_k[None, :]     (matmuls + augmented rows)
  count += sum_j [ PSUM > (sq_q - 0.25) ]   (one fused DVE tensor_scalar
                                             is_gt + row-accumulate)
which is d2 < 0.25 up to fp rounding placement.

Matmul modes:
  f32   - plain fp32 matmuls (4 cycles/row on TensorE).
  f32r  - TF32-like single pass (1 cycle/row): HW rounds both operands to
          ~11-bit mantissas (RNE); d2 error ~2e-6.
  f32r3 - 3-pass split w_h@x_h + w_h@x_l + w_l@x_h with hi = RNE-at-13-bits
          (grid-aligned so the HW's ~12-bit rounding passes hi through
          exactly; 11x11-bit products are exact in fp32) -> fp32-grade
          precision at 3 cycles/row.
The -sq_k augmented rows are always fed as hi+residual pairs so they are
exact under f32r rounding.
"""

import numpy as np

C, P, D = 10, 1000, 1024
N = C * P  # 10000
N_CORES = 8
QT = 125  # query tile rows (divides 1000; 8 tiles per class -> 1 per core)
NQT = C  # query tiles per core (one per class)
QPC = QT * NQT  # 1250 queries per core
KC = 500  # key chunk (matmul free dim)
NKC = N // KC  # 20 chunks; chunk c belongs to class c//2
KS = D // 128  # 8 contraction slices

MM_MODE = "f32r"

_PROG_CACHE = {}


def _rne(a, k):
    """Round fp32 array to (23-k) explicit mantissa bits, round-to-nearest-even."""
    u = a.view(np.uint32).astype(np.uint64)
    bias = ((u >> k) & 1) + np.uint64((1 << (k - 1)) - 1)
    r = (((u + bias) >> k) << k) & np.uint64(0xFFFFFFFF)
    return r.astype(np.uint32).view(np.float32)


def _build_program(mm_mode):
    import concourse.mybir as mybir
    from concourse import bacc
    from concourse.tile import TileContext

    f32 = mybir.dt.float32
    mm_dt = f32 if mm_mode == "f32" else mybir.dt.float32r
    split = mm_mode == "f32r3"
    n_aug = 1 if mm_mode == "f32" else 2

    nc = bacc.Bacc("TRN2", target_bir_lowering=False, debug=False, num_devices=N_CORES)

    # qT holds 2*Q^T k-slices (2x folded into the stationary operand so PSUM
    # accumulates 2G directly). In split mode qT/kT are the hi parts and
    # qTl/kTl the residuals.
    qT = nc.dram_tensor("qT", [KS, 128, QPC], mm_dt, kind="ExternalInput").ap()
    kT = nc.dram_tensor("kT", [KS, 128, N], mm_dt, kind="ExternalInput").ap()
    if split:
        qTl = nc.dram_tensor("qTl", [KS, 128, QPC], mm_dt, kind="ExternalInput").ap()
        kTl = nc.dram_tensor("kTl", [KS, 128, N], mm_dt, kind="ExternalInput").ap()
    kaug = nc.dram_tensor("kaug", [n_aug, N], mm_dt, kind="ExternalInput").ap()
    onesw = nc.dram_tensor("onesw", [n_aug, QT], mm_dt, kind="ExternalInput").ap()
    qthr = nc.dram_tensor("qthr", [QT, NQT], f32, kind="ExternalInput").ap()
    cnt_out = nc.dram_tensor("cnt", [QT, NQT], f32, kind="ExternalOutput").ap()

    with TileContext(nc) as tc:
        with (
            tc.tile_pool(name="qpool", bufs=1) as qpool,
            tc.tile_pool(name="kpool", bufs=2 if split else 3) as kpool,
            tc.tile_pool(name="small", bufs=1) as spool,
            tc.tile_pool(name="psum", bufs=4, space="PSUM") as pspool,
        ):
            qt_tiles = []
            qtl_tiles = []
            for ks in range(KS):
                t = qpool.tile([128, QPC], mm_dt, tag=f"q{ks}")
                nc.sync.dma_start(out=t[:], in_=qT[ks])
                qt_tiles.append(t)
                if split:
                    tl = qpool.tile([128, QPC], mm_dt, tag=f"ql{ks}")
                    nc.sync.dma_start(out=tl[:], in_=qTl[ks])
                    qtl_tiles.append(tl)

            qthr_t = spool.tile([QT, NQT], f32, tag="qthr")
            nc.sync.dma_start(out=qthr_t[:], in_=qthr[:])
            ones_t = spool.tile([n_aug, QT], mm_dt, tag="ones")
            nc.sync.dma_start(out=ones_t[:], in_=onesw[:])

            acc = spool.tile([QT, NQT * 18], f32, tag="acc")
            scratch = spool.tile([QT, KC], f32, tag="scratch")
            cnt_t = spool.tile([QT, NQT], f32, tag="cnt")

            col = [0] * NQT
            for c in range(NKC):
                kt = kpool.tile([128, KS, KC], mm_dt, tag="kt")
                for ks in range(KS):
                    nc.sync.dma_start(
                        out=kt[:, ks, :], in_=kT[ks, :, c * KC : (c + 1) * KC]
                    )
                if split:
                    ktl = kpool.tile([128, KS, KC], mm_dt, tag="ktl")
                    for ks in range(KS):
                        nc.sync.dma_start(
                            out=ktl[:, ks, :], in_=kTl[ks, :, c * KC : (c + 1) * KC]
                        )
                ka = kpool.tile([n_aug, KC], mm_dt, tag="ka")
                nc.sync.dma_start(out=ka[:], in_=kaug[:, c * KC : (c + 1) * KC])

                for t in range(NQT):
                    if t == c // 2:
                        continue  # same-class chunk: reference skips class t
                    ps = pspool.tile([QT, KC], f32)
                    for ks in range(KS):
                        nc.tensor.matmul(
                            ps[:],
                            qt_tiles[ks][:, t * QT : (t + 1) * QT],
                            kt[:, ks, :],
                            start=(ks == 0),
                            stop=False,
                        )
                    if split:
                        for ks in range(KS):
                            nc.tensor.matmul(
                                ps[:],
                                qt_tiles[ks][:, t * QT : (t + 1) * QT],
                                ktl[:, ks, :],
                                start=False,
                                stop=False,
                            )
                        for ks in range(KS):
                            nc.tensor.matmul(
                                ps[:],
                                qtl_tiles[ks][:, t * QT : (t + 1) * QT],
                                kt[:, ks, :],
                                start=False,
                                stop=False,
                            )
                    # augmented row(s): ones^T @ (-sq_k hi/lo) accumulate -sq_j
                    nc.tensor.matmul(ps[:], ones_t[:], ka[:], start=False, stop=True)
                    # count_j [ps > sq_q - 0.25] accumulated along the free axis
                    nc.vector.tensor_scalar(
                        out=scratch[:],
                        in0=ps[:],
                        scalar1=qthr_t[:, t : t + 1],
                        scalar2=None,
                        op0=mybir.AluOpType.is_gt,
                        op1=mybir.AluOpType.add,
                        accum_out=acc[:, t * 18 + col[t] : t * 18 + col[t] + 1],
                    )
                    col[t] += 1

            for t in range(NQT):
                nc.vector.tensor_reduce(
                    cnt_t[:, t : t + 1],
                    acc[:, t * 18 : (t + 1) * 18],
                    axis=mybir.AxisListType.X,
                    op=mybir.AluOpType.add,
                )
            nc.sync.dma_start(out=cnt_out[:], in_=cnt_t[:])

    nc.compile()
    return nc


def _get_program(mm_mode):
    if mm_mode not in _PROG_CACHE:
        _PROG_CACHE[mm_mode] = _build_program(mm_mode)
    return _PROG_CACHE[mm_mode]


def _prepare_inputs(X, sq, mm_mode):
    """Build per-core in_maps from X [N, D] f32 and sq [N] f32."""
    split = mm_mode == "f32r3"

    kT_full = np.ascontiguousarray(X.T.reshape(KS, 128, N))
    if split:
        kT_hi = _rne(kT_full, 13)
        kT_lo = np.ascontiguousarray(kT_full - kT_hi)
        kT_hi = np.ascontiguousarray(kT_hi)

    msq = (-sq).astype(np.float32)
    if mm_mode == "f32":
        kaug_full = msq.reshape(1, N).copy()
    else:
        hi = _rne(msq, 13)
        kaug_full = np.ascontiguousarray(np.stack([hi, msq - hi], axis=0))

    in_maps = []
    for core in range(N_CORES):
        rows = np.concatenate(
            [
                np.arange(cls * P + core * QT, cls * P + (core + 1) * QT)
                for cls in range(C)
            ]
        )
        Q2 = 2.0 * X[rows]  # exact scaling
        qT_c = np.ascontiguousarray(Q2.T.reshape(KS, 128, QPC))
        qthr_c = np.ascontiguousarray(
            (sq[rows] - np.float32(0.25)).reshape(NQT, QT).T
        )
        m = {
            "kT": kT_full,
            "kaug": kaug_full,
            "qthr": qthr_c,
            "onesw": np.ones((kaug_full.shape[0], QT), dtype=np.float32),
        }
        if split:
            qhi = _rne(qT_c, 13)
            m["qT"] = np.ascontiguousarray(qhi)
            m["qTl"] = np.ascontiguousarray(qT_c - qhi)
            m["kT"] = kT_hi
            m["kTl"] = kT_lo
        else:
            m["qT"] = qT_c
        in_maps.append(m)
    return in_maps


def _counts_from_results(results):
    counts = np.zeros(N, dtype=np.int64)
    for core in range(N_CORES):
        out = results[core]["cnt"]  # [QT, NQT] f32
        for cls in range(C):
            counts[cls * P + core * QT : cls * P + (core + 1) * QT] = out[
                :, cls
            ].astype(np.int64)
    return counts


def kernel(feats, ids_per_cls, budget, _bench=None):
    from concourse.bass_utils import run_bass_kernel_spmd

    feats = np.asarray(feats, dtype=np.float32)
    ids_per_cls = np.asarray(ids_per_cls)
    budget_i = int(np.asarray(budget))

    ids_flat = ids_per_cls.reshape(-1).astype(np.int64)
    X = np.ascontiguousarray(feats[ids_flat])  # [N, D] class-blocked
    sq = (X.astype(np.float64) ** 2).sum(axis=1).astype(np.float32)

    nc = _get_program(MM_MODE)
    in_maps = _prepare_inputs(X, sq, MM_MODE)
    kw = dict(_bench) if _bench else {}
    res = run_bass_kernel_spmd(nc, in_maps, core_ids=list(range(N_CORES)), **kw)
    counts = _counts_from_results(res.results)

    counts = counts.reshape(C, P)
    per_cls_budget = budget_i // C
    order = np.argsort(counts, axis=-1, kind="stable")
    sel = order[:, :per_cls_budget]
    ids_selected = np.take_along_axis(
        ids_per_cls.reshape(C, P), sel, axis=1
    ).reshape(-1)

    counts_out = counts.astype(np.int32)
    if _bench is not None:
        return (ids_selected, counts_out), res
    return ids_selected, counts_out
